# revision 1
# baseline (speedup 1.0000x reference)
"""Linformer self-attention block on 8 Trainium2 NeuronCores.

Data-parallel SPMD: the flattened batch b = B*l = 16 is split 2 per core.
Math (per batch, n=4096, c=512, h=8 heads, dh=64, k=256):
    q  = x @ Wq
    xk = proj_k^T @ x            (Linformer folding: proj commutes with Wk)
    xv = proj_v^T @ x
    kp = xk @ Wk ;  vp = xv @ Wv
    dots_h  = (q_h @ kp_h^T) / 8
    attn    = softmax(dots, axis=k)   [no max-subtraction: |dots| < ~6]
    o_h     = attn_h @ vp_h
    y  = o @ Wo + bo

On-chip layout: everything contracting over c uses x^T (PE-transposed
tiles); softmax runs in the (k-partition, n-free) orientation so the
k-contraction of attn@vp needs no attn transpose. Per-head row sums are
materialized pre-broadcast via zero-padded all-ones lhsT matmuls;
normalization folds into the PSUM->SBUF evacuation. Matmuls run as
float32r (TF32: 1 cycle/row vs fp32's 4). fp32r matmuls don't support
tile_position, and engines can't move data across partitions, so
per-head (dh=64) operands are zero-padded to K=128 / M=128 and head
pairs accumulate into one PSUM tile.
"""
import os
import sys

sys.path.insert(0, "/opt/trn_rl_repo")

KSTAGE = os.environ.get("KSTAGE", "full")  # debug: "a", "b1", "full"
KSUB = os.environ.get("KSUB", "full")      # debug: "x", "q", "d", "o", "full"

import numpy as np
import concourse.bass as bass
import concourse.tile as tile
from concourse import bacc, masks, mybir
from concourse.bass_utils import run_bass_kernel_spmd

F32 = mybir.dt.float32
F32R = mybir.dt.float32r
AF = mybir.ActivationFunctionType
OP = mybir.AluOpType

B, L, SEQ, DIM = 2, 8, 4096, 512
H, DH, KL = 8, 64, 256
NCORES = 8
BPC = (B * L) // NCORES   # batches per core
NT = SEQ // 128           # 32 row-tiles
NCH = SEQ // 512          # 8 row-chunks
SCALE = float(DH) ** -0.5


def _phase_a(tc, psA, sbA, x, bi, pkv_sb, wk_sb, wv_sb, zeros_sb,
             kpt_pad, vp_pad):
    nc = tc.nc
    # xkvT[c, kk] = sum_n x[n, c] * pkv[n, kk]
    xkv_sb = sbA.tile([128, 4, DIM], F32R, tag="xkv", bufs=1)
    xkv_ps = psA.tile([128, 4, DIM], F32, tag="xkv_ps", bufs=1)
    for nt in range(NT):
        xa = sbA.tile([128, DIM], F32R, tag="xa", bufs=4)
        nc.sync.dma_start(
            xa[:], x[bi, nt * 128:(nt + 1) * 128, :].bitcast(F32R))
        for ct in range(4):
            nc.tensor.matmul(
                xkv_ps[:, ct, :],
                xa[:, ct * 128:(ct + 1) * 128],
                pkv_sb[:, nt, :],
                start=(nt == 0), stop=(nt == NT - 1))
    for ct in range(4):
        nc.vector.tensor_copy(xkv_sb[:, ct, :], xkv_ps[:, ct, :])

    # kpT[d, k] = sum_c Wk[c, d] * xkT[c, k]; zero-padded per head:
    # kpt_pad[:, h, kt2, :] is (128, 128) with head h's (64, 128) block at
    # partitions (h%2)*64.. and zeros elsewhere.
    kpt_ps = psA.tile([128, 4, KL], F32, tag="kpt_ps", bufs=1)
    for dt in range(4):
        for cc in range(4):
            nc.tensor.matmul(
                kpt_ps[:, dt, :],
                wk_sb[:, cc, dt * 128:(dt + 1) * 128],
                xkv_sb[:, cc, 0:KL],
                start=(cc == 0), stop=(cc == 3))
    nc.vector.tensor_copy(kpt_pad[:], zeros_sb[:, 0:2048])
    for h in range(H):
        rs = slice((h % 2) * 64, (h % 2) * 64 + 64)
        for kt2 in range(2):
            nc.vector.tensor_copy(
                kpt_pad[rs, h, kt2, :],
                kpt_ps[rs, h // 2, kt2 * 128:(kt2 + 1) * 128])

    # vp[k, d] = sum_c xvT[c, k] * Wv[c, d]; zero-padded per head:
    # vp_pad[:, h, kt2, :] is (128, 128) with vp cols h*64.. placed at
    # free offset (h%2)*64 and zeros elsewhere.
    vp_ps = psA.tile([128, 2, DIM], F32, tag="vp_ps", bufs=1)
    for kt2 in range(2):
        for cc in range(4):
            nc.tensor.matmul(
                vp_ps[:, kt2, :],
                xkv_sb[:, cc, KL + kt2 * 128:KL + (kt2 + 1) * 128],
                wv_sb[:, cc, :],
                start=(cc == 0), stop=(cc == 3))
    nc.vector.tensor_copy(vp_pad[:], zeros_sb[:, 0:2048])
    for h in range(H):
        fs = slice((h % 2) * 64, (h % 2) * 64 + 64)
        for kt2 in range(2):
            nc.vector.tensor_copy(
                vp_pad[:, h, kt2, fs],
                vp_ps[:, kt2, h * 64:(h + 1) * 64])


def _phase_b(tc, psB, sbB, x, y, bi, consts, kpt_pad, vp_pad):
    nc = tc.nc
    wq_sb, wo_sb, ident2, ones_pad, bo_bcast = consts
    nch = 1 if KSTAGE == "b1" else NCH
    for nj in range(nch):
        ns = slice(nj * 512, (nj + 1) * 512)
        xb = sbB.tile([128, 4, DIM], F32R, tag="xb", bufs=2)
        xsrc = x[bi, ns, :].rearrange("(ntl p) c -> p ntl c", p=128)
        nc.sync.dma_start(xb[:], xsrc.bitcast(F32R))

        # x^T chunk via identity matmuls: out = x_tile^T @ [I|I].
        # (PE transpose-mode next to regular matmuls wedges the device, so
        # transpose with a plain matmul; N=256 keeps f32r at 1 cyc/row.)
        xt_sb = sbB.tile([128, 4, 512], F32R, tag="xt", bufs=2)
        for ct in range(4):
            xt_ps = psB.tile([128, 4, 256], F32, tag="xtps", bufs=1)
            for ntl in range(4):
                nc.tensor.matmul(
                    xt_ps[:, ntl, :],
                    xb[:, ntl, ct * 128:(ct + 1) * 128],
                    ident2[:],
                    start=True, stop=True)
            nc.vector.tensor_copy(xt_sb[:, ct, :], xt_ps[:, :, 0:128])

        if KSUB == "x":
            continue
        # qT[d, n] = sum_c Wq[c, d] * xT[c, n]   (evac on ACT)
        qt_sb = sbB.tile([128, 4, 512], F32R, tag="qt", bufs=2)
        for dt in range(4):
            qt_ps = psB.tile([128, 512], F32, tag="qtps", bufs=1)
            for cc in range(4):
                nc.tensor.matmul(
                    qt_ps[:],
                    wq_sb[:, cc, dt * 128:(dt + 1) * 128],
                    xt_sb[:, cc, :],
                    start=(cc == 0), stop=(cc == 3))
            if os.environ.get("KQT_ACT"):
                nc.scalar.copy(qt_sb[:, dt, :], qt_ps[:])
            else:
                nc.vector.tensor_copy(qt_sb[:, dt, :], qt_ps[:])

        if KSUB == "q":
            continue
        ot_sb = sbB.tile([128, 4, 512], F32R, tag="ot", bufs=2)
        for hp in range(4):
            # dotsT_h[k, n] = sum_dh kpT_h[dh, k] * qT_h[dh, n]
            # (zero-padded lhsT kills the other head's rows of qT)
            exp_tiles = []
            for hi in range(2):
                h = 2 * hp + hi
                dots_ps = psB.tile([128, 2, 512], F32, tag="dots", bufs=1,
                                   name=f"dots_ps{h}")
                for kt2 in range(2):
                    nc.tensor.matmul(
                        dots_ps[:, kt2, :],
                        kpt_pad[:, h, kt2, :],
                        qt_sb[:, hp, :],
                        start=True, stop=True)
                exp_sb = sbB.tile([128, 2, 512], F32R, tag="exp", bufs=3,
                                  name=f"exp_sb{h}")
                if os.environ.get("KEXP_SPLIT"):
                    for kt2 in range(2):
                        nc.scalar.activation(
                            exp_sb[:, kt2, :], dots_ps[:, kt2, :],
                            AF.Exp, scale=SCALE)
                else:
                    nc.scalar.activation(exp_sb[:], dots_ps[:], AF.Exp,
                                         scale=SCALE)
                exp_tiles.append(exp_sb)

            if KSUB == "d":
                continue
            # oT pair tile: head hi's zero-padded vp lands its (64, n)
            # block at partitions hi*64..; the pair accumulates in PSUM.
            # Same trick with padded ones gives pre-broadcast row sums.
            os_ps = psB.tile([128, 2, 512], F32, tag="os", bufs=1)
            for hi in range(2):
                h = 2 * hp + hi
                for kt2 in range(2):
                    nc.tensor.matmul(
                        os_ps[:, 0, :],
                        vp_pad[:, h, kt2, :],
                        exp_tiles[hi][:, kt2, :],
                        start=(hi == 0 and kt2 == 0),
                        stop=(hi == 1 and kt2 == 1))
            for hi in range(2):
                for kt2 in range(2):
                    nc.tensor.matmul(
                        os_ps[:, 1, :],
                        ones_pad[:, hi, :],
                        exp_tiles[hi][:, kt2, :],
                        start=(hi == 0 and kt2 == 0),
                        stop=(hi == 1 and kt2 == 1))
            rec = sbB.tile([128, 512], F32, tag="rec", bufs=2)
            nc.vector.reciprocal(rec[:], os_ps[:, 1, :])
            nc.vector.scalar_tensor_tensor(
                ot_sb[:, hp, :], os_ps[:, 0, :], 1.0, rec[:],
                op0=OP.mult, op1=OP.mult)

        if KSUB in ("d", "o"):
            continue
        # y[n, d] = sum_do oT[do, n] * Wo[do, d] + bo
        yo_sb = sbB.tile([128, 4, DIM], F32, tag="yo", bufs=2)
        for ntl in range(4):
            y_ps = psB.tile([128, 512], F32, tag="y", bufs=1)
            for hp in range(4):
                nc.tensor.matmul(
                    y_ps[:],
                    ot_sb[:, hp, ntl * 128:(ntl + 1) * 128],
                    wo_sb[:, hp, :],
                    start=(hp == 0), stop=(hp == 3))
            nc.vector.scalar_tensor_tensor(
                yo_sb[:, ntl, :], y_ps[:], 1.0, bo_bcast[:],
                op0=OP.mult, op1=OP.add)
        nc.sync.dma_start(
            y[bi, ns, :].rearrange("(ntl p) c -> p ntl c", p=128),
            yo_sb[:])


def _body(tc, ctx, x, wq, wk, wv, wo, pkv, bo, y):
    nc = tc.nc
    const = ctx.enter_context(tc.tile_pool(name="const", bufs=1))
    sb = ctx.enter_context(tc.tile_pool(name="sb", bufs=1))

    # ---- resident weights (fp32 bits read as f32r; HW truncates) ----
    wq_sb = const.tile([128, 4, DIM], F32R)
    wk_sb = const.tile([128, 4, DIM], F32R)
    wv_sb = const.tile([128, 4, DIM], F32R)
    wo_sb = const.tile([128, 4, DIM], F32R)
    for t, d in ((wq_sb, wq), (wk_sb, wk), (wv_sb, wv), (wo_sb, wo)):
        nc.sync.dma_start(t[:], d.rearrange("(cc p) d -> p cc d", p=128).bitcast(F32R))

    ident_st = const.tile([128, 128], F32)
    masks.make_identity(nc, ident_st[:])
    ident2 = const.tile([128, 256], F32R)
    nc.vector.tensor_copy(ident2[:, 0:128], ident_st[:])
    nc.vector.tensor_copy(ident2[:, 128:256], ident_st[:])

    ones_st = const.tile([128, 128], F32)
    nc.vector.memset(ones_st[:], 1.0)
    ones1 = const.tile([1, 128], F32R)
    nc.vector.tensor_copy(ones1[:], ones_st[0:1, :])

    zeros_sb = const.tile([128, 2048], F32)
    nc.vector.memset(zeros_sb[:], 0.0)

    # ones_pad[:, p, :]: all-ones on free cols p*64..(p+1)*64, else zero
    ones_pad = const.tile([128, 2, 128], F32R)
    nc.vector.tensor_copy(ones_pad[:], zeros_sb[:, 0:256])
    for p in range(2):
        nc.vector.tensor_copy(
            ones_pad[:, p, p * 64:(p + 1) * 64], ones_st[:, 0:64])

    bo_st = const.tile([1, DIM], F32)
    nc.sync.dma_start(bo_st[:], bo[:])
    bo_row = const.tile([1, DIM], F32R)
    nc.vector.tensor_copy(bo_row[:], bo_st[:])
    bo_bcast = const.tile([128, DIM], F32)

    # per-batch Linformer products, alive across phases (zero-padded)
    kpt_pad = [sb.tile([128, H, 2, 128], F32R, tag=f"kpt{i}", name=f"kpt{i}")
               for i in range(BPC)]
    vp_pad = [sb.tile([128, H, 2, 128], F32R, tag=f"vp{i}", name=f"vp{i}")
              for i in range(BPC)]

    # ---- phase A for all batches (pkv resident only here) ----
    with (
        tc.tile_pool(name="sbPKV", bufs=1, space="SBUF") as sbPKV,
        tc.tile_pool(name="psA", bufs=1, space="PSUM") as psA,
        tc.tile_pool(name="sbA", bufs=1, space="SBUF") as sbA,
    ):
        pkv_sb = sbPKV.tile([128, NT, DIM], F32R)
        for nt in range(NT):
            nc.sync.dma_start(
                pkv_sb[:, nt, :],
                pkv[nt * 128:(nt + 1) * 128, :].bitcast(F32R))

        # Pre-touch DMA-resident tensors with throwaway matmuls so real
        # matmuls keep few sync waits.
        junk = psA.tile([128, 256], F32, tag="kpt_ps", bufs=1)
        for t_ap in (wq_sb[:, 0, 0:128], wk_sb[:, 0, 0:128],
                     wv_sb[:, 0, 0:128], wo_sb[:, 0, 0:128]):
            nc.tensor.matmul(junk[:], t_ap, ident2[:], start=True, stop=True)
        bo_ps = psA.tile([128, DIM], F32, tag="vp_ps", bufs=1)
        nc.tensor.matmul(bo_ps[:], ones1[:], bo_row[:], start=True, stop=True)
        nc.vector.tensor_copy(bo_bcast[:], bo_ps[:])

        for bi in range(BPC):
            _phase_a(tc, psA, sbA, x, bi, pkv_sb, wk_sb, wv_sb, zeros_sb,
                     kpt_pad[bi], vp_pad[bi])

    # ---- phase B for all batches ----
    if KSTAGE == "a":
        return
    consts = (wq_sb, wo_sb, ident2, ones_pad, bo_bcast)
    with (
        tc.tile_pool(name="psB", bufs=1, space="PSUM") as psB,
        tc.tile_pool(name="sbB", bufs=1, space="SBUF") as sbB,
    ):
        for bi in range(BPC):
            _phase_b(tc, psB, sbB, x, y, bi, consts, kpt_pad[bi], vp_pad[bi])


def _build():
    from contextlib import ExitStack
    nc = bacc.Bacc("TRN2", target_bir_lowering=False, debug=False,
                   num_devices=NCORES)
    x = nc.declare_dram_parameter("x", [BPC, SEQ, DIM], F32, isOutput=False)
    wq = nc.declare_dram_parameter("wq", [DIM, DIM], F32, isOutput=False)
    wk = nc.declare_dram_parameter("wk", [DIM, DIM], F32, isOutput=False)
    wv = nc.declare_dram_parameter("wv", [DIM, DIM], F32, isOutput=False)
    wo = nc.declare_dram_parameter("wo", [DIM, DIM], F32, isOutput=False)
    pkv = nc.declare_dram_parameter("pkv", [SEQ, 2 * KL], F32, isOutput=False)
    bo = nc.declare_dram_parameter("bo", [1, DIM], F32, isOutput=False)
    y = nc.declare_dram_parameter("y", [BPC, SEQ, DIM], F32, isOutput=True)
    with tile.TileContext(nc) as tc, ExitStack() as ctx:
        _body(tc, ctx, x, wq, wk, wv, wo, pkv, bo, y)
    nc.compile()
    return nc


_prog = None


def _get_prog():
    global _prog
    if _prog is None:
        _prog = _build()
    return _prog


def kernel(x, Wq, Wk, Wv, proj_k, proj_v, Wo, bo, _trace=False):
    x = np.ascontiguousarray(x, dtype=np.float32).reshape(B * L, SEQ, DIM)
    pkv = np.ascontiguousarray(
        np.concatenate([np.asarray(proj_k), np.asarray(proj_v)], axis=1),
        dtype=np.float32)
    wq = np.ascontiguousarray(Wq, dtype=np.float32)
    wk = np.ascontiguousarray(Wk, dtype=np.float32)
    wv = np.ascontiguousarray(Wv, dtype=np.float32)
    wo = np.ascontiguousarray(Wo, dtype=np.float32)
    bo2 = np.ascontiguousarray(bo, dtype=np.float32).reshape(1, DIM)

    in_maps = [
        {"x": x[c * BPC:(c + 1) * BPC], "wq": wq, "wk": wk, "wv": wv,
         "wo": wo, "pkv": pkv, "bo": bo2}
        for c in range(NCORES)
    ]
    res = run_bass_kernel_spmd(
        _get_prog(), in_maps, core_ids=list(range(NCORES)), trace=_trace)
    out = np.concatenate([res.results[c]["y"] for c in range(NCORES)], axis=0)
    if _trace:
        kernel._last = res
    return out.reshape(B, L, SEQ, DIM)



# revision 19
# speedup vs baseline: 5.5111x; 5.5111x over previous
"""Linformer self-attention block on 8 Trainium2 NeuronCores — bf16 v2.

Data-parallel SPMD: the flattened batch b = B*l = 16 is split 2 per core.
Math (per batch, n=4096, c=512, h=8 heads, dh=64, k=256):
    q  = x @ Wq
    xk = proj_k^T @ x            (Linformer folding: proj commutes with Wk)
    xv = proj_v^T @ x
    kp = xk @ Wk ;  vp = xv @ Wv
    dots_h  = (q_h @ kp_h^T) / 8
    attn    = softmax(dots, axis=k)   [no max-subtraction: |dots| < ~6]
    o_h     = attn_h @ vp_h
    y  = o @ Wo + bo

v2 vs v1: all matmuls in bf16 (rel err ~8.5e-3, tol 2e-2), which buys:
  - x is uploaded in BOTH orientations (x and x^T, bf16) so phase B's
    x^T tiles come from plain DMAs — the on-chip PE-transpose is gone.
  - dots runs as per-head K=64 row-tiles at base partitions 0/64: the
    two heads of a pair execute concurrently in disjoint PE row groups
    (f32r needed zero-padded K=128 lhsT, serializing the pair).
  - attn@v and the softmax row-sum matmuls run as M=64 col-tiles at
    out partitions 0/64 — again pairwise-concurrent.
  - HBM traffic roughly halves.
Row sums are materialized pre-broadcast with an all-ones [128,64] lhsT
(engines can't broadcast across partitions); normalization is
reciprocal + scalar_tensor_tensor into the PSUM->SBUF evacuation.
"""
import os
import sys

sys.path.insert(0, "/opt/trn_rl_repo")

KSTAGE = os.environ.get("KSTAGE", "full")  # debug: "a", "full"

import numpy as np
import ml_dtypes
import concourse.bass as bass
import concourse.tile as tile
from concourse import bacc, mybir
from concourse.bass_utils import run_bass_kernel_spmd

F32 = mybir.dt.float32
F32R = mybir.dt.float32r
BF16 = mybir.dt.bfloat16
AF = mybir.ActivationFunctionType
OP = mybir.AluOpType

B, L, SEQ, DIM = 2, 8, 4096, 512
H, DH, KL = 8, 64, 256
NCORES = 8
BPC = (B * L) // NCORES   # batches per core
NT = SEQ // 128           # 32 row-tiles
NCH = SEQ // 512          # 8 row-chunks
SCALE = float(DH) ** -0.5
NPBF = ml_dtypes.bfloat16


def _phase_a(tc, ps, sb, x, bi, pkv_sb, w_sb, kpt_sb, vp_sb):
    """Per-batch Linformer fold: xkvT = x^T @ [proj_k|proj_v], then
    kpT = Wk^T @ xkT (kept [d-pair, kt2, k]) and vp = xvT^T @ Wv
    (kept [k, kt2, d])."""
    nc = tc.nc
    # x tiles for this batch: 8 chunks of [128, 4, 512] bf16 (1 MB DMAs),
    # all resident until the 4 ct-passes below are done.
    xa = [sb.tile([128, 4, DIM], BF16, tag=f"xa{i}", name=f"xa{bi}_{i}")
          for i in range(8)]
    for i in range(8):
        nc.sync.dma_start(
            xa[i][:],
            x[bi, i * 512:(i + 1) * 512, :].rearrange(
                "(ntl p) c -> p ntl c", p=128))

    # xkvT[c, kk] = sum_n x[n, c] * pkv[n, kk]; one ct-slice (128 c rows)
    # per PSUM bank pass.
    xkv_sb = sb.tile([128, 4, DIM], BF16, tag="xkv", bufs=2)
    for ct in range(4):
        xkv_ps = ps.tile([128, DIM], F32, tag="dots", bufs=4)
        for i in range(8):
            for j in range(4):
                nt = i * 4 + j
                nc.tensor.matmul(
                    xkv_ps[:],
                    xa[i][:, j, ct * 128:(ct + 1) * 128],
                    pkv_sb[nt // 8][:, nt % 8, :],
                    start=(i == 0 and j == 0), stop=(i == 7 and j == 3))
        nc.vector.tensor_copy(xkv_sb[:, ct, :], xkv_ps[:])

    # kpT[d, k] = sum_c Wk[c, d] * xkT[c, k]; dt == head-pair hp.
    for hp in range(4):
        kpt_ps = ps.tile([128, KL], F32, tag="dots", bufs=4)
        for cc in range(4):
            nc.tensor.matmul(
                kpt_ps[:],
                w_sb[:, cc, DIM + hp * 128:DIM + (hp + 1) * 128],
                xkv_sb[:, cc, 0:KL],
                start=(cc == 0), stop=(cc == 3))
        nc.vector.tensor_copy(kpt_sb[:, hp, :], kpt_ps[:])

    # vp[k, d] = sum_c xvT[c, k] * Wv[c, d]
    for kt2 in range(2):
        vp_ps = ps.tile([128, DIM], F32, tag="dots", bufs=4)
        for cc in range(4):
            nc.tensor.matmul(
                vp_ps[:],
                xkv_sb[:, cc, KL + kt2 * 128:KL + (kt2 + 1) * 128],
                w_sb[:, cc, 2 * DIM:3 * DIM],
                start=(cc == 0), stop=(cc == 3))
        nc.vector.tensor_copy(vp_sb[:, kt2, :], vp_ps[:])


def _phase_b(tc, ps, sb, xt, y, bi, consts, kpt_sb, vp_sb):
    nc = tc.nc
    w_sb, ones_bf, bo_bcast = consts
    for nj in range(NCH):
        ns = slice(nj * 512, (nj + 1) * 512)
        # x^T chunk [128, 4(cc), 512] straight from the transposed upload.
        xt_t = sb.tile([128, 4, 512], BF16, tag="xt", bufs=3)
        nc.sync.dma_start(
            xt_t[:],
            xt[bi, :, ns].rearrange("(cc p) n -> p cc n", p=128))

        # qT[d, n] = sum_c Wq[c, d] * xT[c, n]; dt == head-pair hp.
        qt_sb = []
        for hp in range(4):
            qt_ps = ps.tile([128, 512], F32, tag="qt", bufs=2)
            for cc in range(4):
                nc.tensor.matmul(
                    qt_ps[:],
                    w_sb[:, cc, hp * 128:(hp + 1) * 128],
                    xt_t[:, cc, :],
                    start=(cc == 0), stop=(cc == 3))
            qt = sb.tile([128, 512], BF16, tag="qtsb", bufs=8,
                         name=f"qt{bi}_{nj}_{hp}")
            nc.vector.tensor_copy(qt[:], qt_ps[:])
            qt_sb.append(qt)

        ot_sb = []
        for hp in range(4):
            # dotsT_h[k, n] = sum_dh kpT_h[dh, k] * qT_h[dh, n]
            # Per-head K=64 row-tiles at partitions 0/64. kt2-outer issue
            # order keeps consecutive matmuls in DISJOINT PE row groups so
            # the head pair executes concurrently (strict-FIFO issue).
            exp_tiles = [
                sb.tile([128, 2, 512], BF16, tag="exp", bufs=4,
                        name=f"exp{bi}_{nj}_{2*hp+hi}")
                for hi in range(2)]
            for kt2 in range(2):
                for hi in range(2):
                    rs = slice(hi * 64, hi * 64 + 64)
                    dots_ps = ps.tile([128, 512], F32, tag="dots", bufs=4)
                    nc.tensor.matmul(
                        dots_ps[:],
                        kpt_sb[rs, hp, kt2 * 128:(kt2 + 1) * 128],
                        qt_sb[hp][rs, :],
                        start=True, stop=True)
                    nc.scalar.activation(
                        exp_tiles[hi][:, kt2, :], dots_ps[:], AF.Exp,
                        scale=SCALE)

            # o pair tile: head hi's M=64 col-tile lands at out partitions
            # hi*64 (pairwise-concurrent col groups); row sums via all-ones
            # lhsT to the same split.
            os_ps = ps.tile([128, 512], F32, tag="os", bufs=1)
            den_ps = ps.tile([128, 512], F32, tag="den", bufs=1)
            for kt2 in range(2):
                for hi in range(2):
                    rs = slice(hi * 64, hi * 64 + 64)
                    nc.tensor.matmul(
                        os_ps[rs, :],
                        vp_sb[:, kt2, hp * 128 + hi * 64:
                              hp * 128 + hi * 64 + 64],
                        exp_tiles[hi][:, kt2, :],
                        start=(kt2 == 0), stop=(kt2 == 1))
            for kt2 in range(2):
                for hi in range(2):
                    rs = slice(hi * 64, hi * 64 + 64)
                    nc.tensor.matmul(
                        den_ps[rs, :],
                        ones_bf[:],
                        exp_tiles[hi][:, kt2, :],
                        start=(kt2 == 0), stop=(kt2 == 1))
            rec = sb.tile([128, 512], F32, tag="rec", bufs=2)
            nc.vector.reciprocal(rec[:], den_ps[:])
            ot = sb.tile([128, 512], BF16, tag="ot", bufs=8,
                         name=f"ot{bi}_{nj}_{hp}")
            nc.vector.scalar_tensor_tensor(
                ot[:], os_ps[:], 1.0, rec[:], op0=OP.mult, op1=OP.mult)
            ot_sb.append(ot)

        # y[n, d] = sum_do oT[do, n] * Wo[do, d] + bo
        yo_sb = sb.tile([128, 4, DIM], F32, tag="yo", bufs=2)
        for ntl in range(4):
            y_ps = ps.tile([128, 512], F32, tag="qt", bufs=2)
            for hp in range(4):
                nc.tensor.matmul(
                    y_ps[:],
                    ot_sb[hp][:, ntl * 128:(ntl + 1) * 128],
                    w_sb[:, hp, 3 * DIM:4 * DIM],
                    start=(hp == 0), stop=(hp == 3))
            nc.vector.scalar_tensor_tensor(
                yo_sb[:, ntl, :], y_ps[:], 1.0, bo_bcast[:],
                op0=OP.mult, op1=OP.add)
        nc.sync.dma_start(
            y[bi, ns, :].rearrange("(ntl p) c -> p ntl c", p=128),
            yo_sb[:])


def _body(tc, ctx, x, xt, w, pkv, bo, y):
    nc = tc.nc
    const = ctx.enter_context(tc.tile_pool(name="const", bufs=1))
    sb = ctx.enter_context(tc.tile_pool(name="sb", bufs=1))
    ps = ctx.enter_context(tc.tile_pool(name="ps", bufs=1, space="PSUM"))

    # ---- resident constants ----
    # w packs [Wq | Wk | Wv | Wo] along the output dim: [512 c, 2048].
    w_sb = const.tile([128, 4, 4 * DIM], BF16)
    nc.sync.dma_start(w_sb[:], w.rearrange("(cc p) d -> p cc d", p=128))
    pkv_t = [const.tile([128, 8, DIM], BF16, name=f"pkv{i}")
             for i in range(4)]
    for i in range(4):
        nc.sync.dma_start(
            pkv_t[i][:],
            pkv[i * 1024:(i + 1) * 1024, :].rearrange(
                "(nt p) k -> p nt k", p=128))

    ones_st = const.tile([128, 128], F32)
    nc.vector.memset(ones_st[:], 1.0)
    ones_bf = const.tile([128, 64], BF16)
    nc.vector.tensor_copy(ones_bf[:], ones_st[:, 0:64])
    ones1 = const.tile([1, 128], F32R)
    nc.vector.tensor_copy(ones1[:], ones_st[0:1, :])

    bo_st = const.tile([1, DIM], F32)
    nc.sync.dma_start(bo_st[:], bo[:])
    bo_row = const.tile([1, DIM], F32R)
    nc.vector.tensor_copy(bo_row[:], bo_st[:])
    bo_bcast = const.tile([128, DIM], F32)
    bo_ps = ps.tile([128, DIM], F32, tag="dots", bufs=4)
    nc.tensor.matmul(bo_ps[:], ones1[:], bo_row[:], start=True, stop=True)
    nc.vector.tensor_copy(bo_bcast[:], bo_ps[:])

    # per-batch Linformer products, alive from phase A to end of phase B
    kpt_sb = [const.tile([128, 4, KL], BF16, name=f"kpt{i}")
              for i in range(BPC)]
    vp_sb = [const.tile([128, 2, DIM], BF16, name=f"vp{i}")
             for i in range(BPC)]

    consts = (w_sb, ones_bf, bo_bcast)
    for bi in range(BPC):
        _phase_a(tc, ps, sb, x, bi, pkv_t, w_sb, kpt_sb[bi], vp_sb[bi])
        if KSTAGE != "a":
            _phase_b(tc, ps, sb, xt, y, bi, consts, kpt_sb[bi], vp_sb[bi])


def _build():
    from contextlib import ExitStack
    nc = bacc.Bacc("TRN2", target_bir_lowering=False, debug=False,
                   num_devices=NCORES)
    x = nc.declare_dram_parameter("x", [BPC, SEQ, DIM], BF16, isOutput=False)
    xt = nc.declare_dram_parameter("xt", [BPC, DIM, SEQ], BF16, isOutput=False)
    w = nc.declare_dram_parameter("w", [DIM, 4 * DIM], BF16, isOutput=False)
    pkv = nc.declare_dram_parameter("pkv", [SEQ, 2 * KL], BF16, isOutput=False)
    bo = nc.declare_dram_parameter("bo", [1, DIM], F32, isOutput=False)
    y = nc.declare_dram_parameter("y", [BPC, SEQ, DIM], F32, isOutput=True)
    with tile.TileContext(nc) as tc, ExitStack() as ctx:
        _body(tc, ctx, x, xt, w, pkv, bo, y)
    nc.compile()
    return nc


_prog = None


def _get_prog():
    global _prog
    if _prog is None:
        _prog = _build()
    return _prog


def make_per_core_inputs(inputs):
    """Host-side prep shared by kernel() and benches: bf16 casts, packed
    weights, and the transposed x upload."""
    x32 = np.asarray(inputs["x"], dtype=np.float32).reshape(B * L, SEQ, DIM)
    xb = np.ascontiguousarray(x32).astype(NPBF)
    xtb = np.ascontiguousarray(x32.transpose(0, 2, 1)).astype(NPBF)
    w = np.concatenate(
        [np.asarray(inputs[k], dtype=np.float32) for k in
         ("Wq", "Wk", "Wv", "Wo")], axis=1).astype(NPBF)
    pkv = np.concatenate(
        [np.asarray(inputs["proj_k"], dtype=np.float32),
         np.asarray(inputs["proj_v"], dtype=np.float32)], axis=1).astype(NPBF)
    bo = np.asarray(inputs["bo"], dtype=np.float32).reshape(1, DIM)
    return {"x": xb, "xt": xtb, "w": np.ascontiguousarray(w),
            "pkv": np.ascontiguousarray(pkv), "bo": bo}


def kernel(x, Wq, Wk, Wv, proj_k, proj_v, Wo, bo, _trace=False):
    pc = make_per_core_inputs(dict(
        x=x, Wq=Wq, Wk=Wk, Wv=Wv, proj_k=proj_k, proj_v=proj_v, Wo=Wo,
        bo=bo))
    in_maps = [
        {"x": pc["x"][c * BPC:(c + 1) * BPC],
         "xt": pc["xt"][c * BPC:(c + 1) * BPC],
         "w": pc["w"], "pkv": pc["pkv"], "bo": pc["bo"]}
        for c in range(NCORES)
    ]
    res = run_bass_kernel_spmd(
        _get_prog(), in_maps, core_ids=list(range(NCORES)), trace=_trace)
    out = np.concatenate([res.results[c]["y"] for c in range(NCORES)], axis=0)
    if _trace:
        kernel._last = res
    return out.reshape(B, L, SEQ, DIM)
